# revision 1
# baseline (speedup 1.0000x reference)
"""Trainium2 Bass kernel for EquivariantBinaryClassificationNoGraphScalar.

Computation (see reference):
    s[b, c]  = sum_n x[b, n, c]                      # node-sum, N=256
    h        = LayerNorm_C(s) * ln_w + ln_b          # over C=1024
    out[b]   = sigmoid(h . W[0] + b)                 # Linear(C, 1)

Sharding: data-parallel over batch. x is [1024, 256, 1024] f32 (1 GiB);
each of 8 cores gets a [128, 256, 1024] shard (128 MiB) -> memory-bound,
per-core HBM roofline ~128MiB / 358GB/s ~= 375 us.

Per-core algorithm (batch lives on the partition axis, so no
cross-partition reduction is ever needed):
  - Stream x as [128(batch), NCHUNK(node), 1024(chan)] tiles: partition
    stride 1 MiB, NCHUNK*4KiB contiguous per partition.
  - VectorE accumulates acc[b, c] += x[:, n, :] (one 1x-mode f32 add per
    node slice; ~256 * 1024 cycles ~= 190 us, hidden under DMA).
  - Epilogue for all 128 batches at once: bn_stats/bn_aggr -> mu, var;
    rstd = 1/sqrt(var+eps); logits = rstd*(s.wln - mu*sum(wln)) + c0
    with wln = ln_w*W[0], c0 = sum(ln_b*W[0]) + b; sigmoid on ScalarE.
"""

import sys

import numpy as np

if "/opt/trn_rl_repo" not in sys.path:
    sys.path.insert(0, "/opt/trn_rl_repo")

from contextlib import ExitStack

import concourse.bacc as bacc
import concourse.bass as bass
import concourse.tile as tile
from concourse import mybir
from concourse.bass_utils import run_bass_kernel_spmd

B, N, C = 1024, 256, 1024
NCORES = 8
BS = B // NCORES  # 128 batches per core
P = 128
FP32 = mybir.dt.float32
LN_EPS = 1e-5

# 1 MiB per dma_start (128 descriptors x 8 KiB) measured fastest on HW:
# ~110 us/pass better than 2 MiB chunks, and 4 MiB+ chunks are worse
# still. 0.5 MiB chunks regress (per-instruction HWDGE overhead, 256
# instructions on one ring). Splitting across the SP+Act HWDGE rings
# does not help at any chunk size (both rings feed the same 16 SDMA
# engines).
NCHUNK = 2  # node slices per DMA -> 1 MiB per transfer
X_BUFS = 12

# Kept for test.py: the BassKernelResults of the last kernel() call
# (exec_time_ns is populated when BASS_TRACE=1).
LAST_RESULT = None


def build(
    bs: int = BS,
    nchunk: int = NCHUNK,
    x_bufs: int = X_BUFS,
    passes: int = 1,
    queues: str = "s",
):
    """Build the per-core Bass module. bs<128 gives a small variant for sim.

    passes>1 streams x that many times (acc reset each pass; result
    unchanged) — used by test.py to measure pure device time per pass as
    slope(passes=2) - slope(passes=1).

    queues: per-chunk DMA trigger queue, cycled. "s"=SP HWDGE,
    "a"=Activation HWDGE, "g"=Pool SWDGE (gpsimd). e.g. "sa" alternates
    the two hardware DGE queues to use more DMA engines in parallel.
    """
    # Bacc (not raw Bass): its finalize() runs generate_event_semaphores,
    # which splits multi-sem waits (TRN2 allows 1 sync wait per instruction).
    nc = bacc.Bacc(None)
    x = nc.declare_dram_parameter("x", [bs, N, C], FP32, isOutput=False)
    ln_w = nc.declare_dram_parameter("ln_w", [C], FP32, isOutput=False)
    ln_b = nc.declare_dram_parameter("ln_b", [C], FP32, isOutput=False)
    W = nc.declare_dram_parameter("W", [1, C], FP32, isOutput=False)
    bias = nc.declare_dram_parameter("b", [1], FP32, isOutput=False)
    out = nc.declare_dram_parameter("out", [bs, 1], FP32, isOutput=True)

    with tile.TileContext(nc) as tc, ExitStack() as ctx:
        xpool = ctx.enter_context(tc.tile_pool(name="xp", bufs=x_bufs))
        singles = ctx.enter_context(tc.tile_pool(name="si", bufs=1))
        ep = ctx.enter_context(tc.tile_pool(name="ep", bufs=1))

        eps_t = singles.tile([P, 1], FP32)
        nc.vector.memset(eps_t, LN_EPS)

        def bcast_load(src_ap, ncols, name):
            """Replicate a [ncols] DRAM vector across all partitions."""
            t = singles.tile([P, ncols], FP32, name=name)
            bc = bass.AP(
                tensor=src_ap.tensor,
                offset=src_ap.offset,
                ap=[[0, P]] + [list(d) for d in src_ap.ap],
            )
            nc.gpsimd.dma_start(out=t, in_=bc)
            return t

        lnw_t = bcast_load(ln_w[:], C, "lnw_t")
        lnb_t = bcast_load(ln_b[:], C, "lnb_t")
        w_t = bcast_load(W[0], C, "w_t")
        b_t = bcast_load(bias[:], 1, "b_t")

        # ---- main loop: acc[b, c] = sum_n x[b, n, c] ----
        qmap = {"s": nc.sync, "a": nc.scalar, "g": nc.gpsimd}
        acc = singles.tile([P, C], FP32)
        for _ in range(passes):
            nc.vector.memset(acc[:bs], 0.0)
            for idx, n0 in enumerate(range(0, N, nchunk)):
                xt = xpool.tile([P, nchunk, C], FP32)
                eng = qmap[queues[idx % len(queues)]]
                eng.dma_start(out=xt[:bs], in_=x[:, n0 : n0 + nchunk, :])
                for j in range(nchunk):
                    nc.vector.tensor_add(acc[:bs], acc[:bs], xt[:bs, j, :])

        # ---- epilogue: all `bs` batches at once, partition = batch ----
        s = acc
        stats = ep.tile([P, 2, 6], FP32)
        sv = s.rearrange("p (g d) -> p g d", g=2)
        for g in range(2):
            nc.vector.bn_stats(out=stats[:bs, g, :], in_=sv[:bs, g, :])
        mv = ep.tile([P, 2], FP32)
        nc.vector.bn_aggr(out=mv[:bs], in_=stats[:bs])
        mu = mv[:bs, 0:1]
        var = mv[:bs, 1:2]

        std = ep.tile([P, 1], FP32)
        nc.scalar.activation(
            out=std[:bs],
            in_=var,
            func=mybir.ActivationFunctionType.Sqrt,
            bias=eps_t[:bs],
            scale=1.0,
        )
        rstd = ep.tile([P, 1], FP32)
        nc.vector.reciprocal(out=rstd[:bs], in_=std[:bs])

        # wln = ln_w * W ; swln = sum(wln) ; c0 = sum(ln_b * W) + b
        # (DVE instructions encode at most ONE sync wait, so give each
        # broadcast-DMA'd tile a single-dependency first consumer.)
        wcopy = singles.tile([P, C], FP32)
        nc.vector.tensor_copy(wcopy, w_t)
        wln = singles.tile([P, C], FP32)
        nc.vector.tensor_mul(wln, lnw_t, wcopy)
        swln = ep.tile([P, 1], FP32)
        nc.vector.reduce_sum(out=swln, in_=wln, axis=mybir.AxisListType.X)
        # (tensor_tensor_reduce is avoided: its custom DVE ucode isn't
        # shipped via this compile path and it kills the exec unit.)
        scr0 = ep.tile([P, C], FP32)
        c0 = ep.tile([P, 1], FP32)
        nc.vector.tensor_mul(scr0, lnb_t, wcopy)
        nc.vector.reduce_sum(out=c0, in_=scr0, axis=mybir.AxisListType.X)
        nc.vector.tensor_add(c0, c0, b_t)

        # dot = s . wln  (per batch row)
        scr1 = ep.tile([P, C], FP32)
        dot = ep.tile([P, 1], FP32)
        nc.vector.tensor_mul(scr1[:bs], s[:bs], wln[:bs])
        nc.vector.reduce_sum(out=dot[:bs], in_=scr1[:bs], axis=mybir.AxisListType.X)

        # logits = rstd * (dot - mu * swln); out = sigmoid(logits + c0)
        t0 = ep.tile([P, 1], FP32)
        nc.vector.tensor_mul(t0[:bs], mu, swln[:bs])
        t1 = ep.tile([P, 1], FP32)
        nc.vector.tensor_sub(t1[:bs], dot[:bs], t0[:bs])
        t2 = ep.tile([P, 1], FP32)
        nc.vector.tensor_mul(t2[:bs], t1[:bs], rstd[:bs])
        res = ep.tile([P, 1], FP32)
        nc.scalar.activation(
            out=res[:bs],
            in_=t2[:bs],
            func=mybir.ActivationFunctionType.Sigmoid,
            bias=c0[:bs],
            scale=1.0,
        )
        nc.sync.dma_start(out=out[:, :], in_=res[:bs])

    # Run the Bacc compile pipeline (register allocation + multi-sync-wait
    # splitting via generate_event_semaphores) — nothing else in the
    # run_bass_kernel_spmd/axon path calls finalize for us.
    nc.finalize()
    return nc


_NC_CACHE = {}


def kernel(**inputs) -> np.ndarray:
    global LAST_RESULT
    x = np.ascontiguousarray(np.asarray(inputs["x"], dtype=np.float32))
    ln_w = np.ascontiguousarray(np.asarray(inputs["ln_w"], dtype=np.float32))
    ln_b = np.ascontiguousarray(np.asarray(inputs["ln_b"], dtype=np.float32))
    W = np.ascontiguousarray(np.asarray(inputs["W"], dtype=np.float32))
    b = np.ascontiguousarray(np.asarray(inputs["b"], dtype=np.float32))

    if "full" not in _NC_CACHE:
        _NC_CACHE["full"] = build()
    nc = _NC_CACHE["full"]

    in_maps = [
        {
            "x": x[i * BS : (i + 1) * BS],
            "ln_w": ln_w,
            "ln_b": ln_b,
            "W": W,
            "b": b,
        }
        for i in range(NCORES)
    ]
    res = run_bass_kernel_spmd(nc, in_maps, list(range(NCORES)))
    LAST_RESULT = res
    return np.concatenate([res.results[i]["out"] for i in range(NCORES)], axis=0)



# revision 2
# speedup vs baseline: 1.1970x; 1.1970x over previous
"""Trainium2 Bass kernel for EquivariantBinaryClassificationNoGraphScalar.

Computation (see reference):
    s[b, c]  = sum_n x[b, n, c]                      # node-sum, N=256
    h        = LayerNorm_C(s) * ln_w + ln_b          # over C=1024
    out[b]   = sigmoid(h . W[0] + b)                 # Linear(C, 1)

Sharding: data-parallel over batch. x is [1024, 256, 1024] f32 (1 GiB);
each of 8 cores gets a [128, 256, 1024] shard (128 MiB) -> memory-bound,
per-core HBM roofline ~128MiB / 358GB/s ~= 375 us.

Per-core algorithm ("pebs2r" — batch-pair-sequential PE reduction):
  - HW-measured: HBM streaming collapses for 1 MiB-strided descriptor
    patterns (the batch-on-partition layout: 293 GB/s at 8 KiB/desc,
    ~160 GB/s at 32 KiB/desc), but runs at the full ~340-375 us/pass
    rate (DMA-only probe) when the windows are read *sequentially*.
    So chunk = batch pair (c0, c0+1): one fully contiguous 2 MiB window
    [128p, 4096]; partition p<64 holds batch c0 nodes {4p..4p+3}, p>=64
    batch c0+1 nodes {4(p-64)..} (16 KiB/partition descriptors).
  - The node axis now lives on partitions, so the reduction runs on the
    (otherwise idle) TensorEngine: lhsT columns c0/c0+1 are the
    upper/lower block-one-hot columns (a sliding slice of a [128, 256]
    selector with ones at col 128 for p<64 and col 129 for p>=64), so
    one matmul pass lands both batches' node-residue sums in psum rows
    c0/c0+1. 8 matmuls of 512 (one per PSUM bank) per chunk; psum is
    [128, 4096] (all 8 banks), holding 4 node-residue partial sums per
    channel, combined by one DVE copy + 3 adds at the end.
  - x / xt / sel are float32r (same bits as f32; np-dtype float32): fp32
    matmuls stream at 4 cyc/row (PE-bound, ~437 us/pass), fp32r at
    1 cyc/row (~140 us/pass, hidden under DMA). HW-probed precision:
    ~1e-4 relative on a 128-wide one-hot reduction — far inside the 2e-2
    gate. The birverifier demands f32r-producing instructions (bitcast
    views of f32-produced data are rejected): DMA propagates f32r from
    the f32r-declared DRAM input, and sel is DVE copy-cast from an f32
    staging tile (Memset can't emit f32r directly).
  - psum[:, 0:1024] holds even-node partial sums, psum[:, 1024:2048] odd;
    one DVE copy+add -> s[b, c], then the usual LN/Linear epilogue:
    bn_stats/bn_aggr -> mu, var; logits = rstd*(s.wln - mu*sum(wln)) + c0
    with wln = ln_w*W[0], c0 = sum(ln_b*W[0]) + b; sigmoid on ScalarE.
"""

import sys

import numpy as np

if "/opt/trn_rl_repo" not in sys.path:
    sys.path.insert(0, "/opt/trn_rl_repo")

from contextlib import ExitStack

import concourse.bacc as bacc
import concourse.bass as bass
import concourse.tile as tile
from concourse import mybir
from concourse.bass_utils import run_bass_kernel_spmd

B, N, C = 1024, 256, 1024
NCORES = 8
BS = B // NCORES  # 128 batches per core
P = 128
FP32 = mybir.dt.float32
FP32R = mybir.dt.float32r
LN_EPS = 1e-5

X_BUFS = 8  # 2 MiB tiles x 8 = 16 MiB of DMA lookahead

# Kept for test.py: the BassKernelResults of the last kernel() call.
LAST_RESULT = None


def build(
    bs: int = BS,
    x_bufs: int = X_BUFS,
    passes: int = 1,
    queues: str = "s",
):
    """Build the per-core Bass module.

    passes>1 streams x that many times (psum reset via start=True each
    pass; result unchanged) — used by test.py to measure pure device time
    per pass as slope(passes=hi) - slope(passes=lo).
    """
    assert bs == BS, "window layout assumes full 128-batch shards"
    nc = bacc.Bacc(None)
    x = nc.declare_dram_parameter("x", [bs, N, C], FP32R, isOutput=False)
    ln_w = nc.declare_dram_parameter("ln_w", [C], FP32, isOutput=False)
    ln_b = nc.declare_dram_parameter("ln_b", [C], FP32, isOutput=False)
    W = nc.declare_dram_parameter("W", [1, C], FP32, isOutput=False)
    bias = nc.declare_dram_parameter("b", [1], FP32, isOutput=False)
    out = nc.declare_dram_parameter("out", [bs, 1], FP32, isOutput=True)

    with tile.TileContext(nc) as tc, ExitStack() as ctx:
        xpool = ctx.enter_context(tc.tile_pool(name="xp", bufs=x_bufs))
        singles = ctx.enter_context(tc.tile_pool(name="si", bufs=1))
        ep = ctx.enter_context(tc.tile_pool(name="ep", bufs=1))
        psum_pool = ctx.enter_context(tc.tile_pool(name="ps", bufs=1, space="PSUM"))

        eps_t = singles.tile([P, 1], FP32)
        nc.vector.memset(eps_t, LN_EPS)

        def bcast_load(src_ap, ncols, name):
            """Replicate a [ncols] DRAM vector across all partitions."""
            t = singles.tile([P, ncols], FP32, name=name)
            bc = bass.AP(
                tensor=src_ap.tensor,
                offset=src_ap.offset,
                ap=[[0, P]] + [list(d) for d in src_ap.ap],
            )
            nc.gpsimd.dma_start(out=t, in_=bc)
            return t

        lnw_t = bcast_load(ln_w[:], C, "lnw_t")
        lnb_t = bcast_load(ln_b[:], C, "lnb_t")
        w_t = bcast_load(W[0], C, "w_t")
        b_t = bcast_load(bias[:], 1, "b_t")

        # selector: sel[p, 128] = 1 for p<64, sel[p, 129] = 1 for p>=64;
        # the slice sel[:, 128-c0 : 256-c0] puts those block-one-hot
        # columns at positions c0 / c0+1. Staged in f32 (Memset can't emit
        # f32r), copy-cast to the f32r matmul operand.
        sel_f = singles.tile([P, 256], FP32)
        nc.vector.memset(sel_f, 0.0)
        nc.vector.memset(sel_f[0:64, 128:129], 1.0)
        nc.vector.memset(sel_f[64:128, 129:130], 1.0)
        sel = singles.tile([P, 256], FP32R)
        nc.vector.tensor_copy(sel, sel_f)

        psum = psum_pool.tile([P, 4 * C], FP32)
        acc = singles.tile([P, C], FP32)
        qmap = {"s": nc.sync, "a": nc.scalar, "g": nc.gpsimd}

        # ---- main loop: psum[c, r*C + ch] = node-residue-r sums ----
        for _ in range(passes):
            for c0 in range(0, bs, 2):
                xt = xpool.tile([P, 4 * C], FP32R)
                src = bass.AP(
                    tensor=x[:, :, :].tensor,
                    offset=c0 * N * C,
                    ap=[[4 * C, P], [1, 4 * C]],
                )
                eng = qmap[queues[(c0 // 2) % len(queues)]]
                eng.dma_start(out=xt, in_=src)
                w = sel[:, 128 - c0 : 256 - c0]
                for q in range(8):
                    nc.tensor.matmul(
                        psum[:, q * 512 : (q + 1) * 512],
                        w,
                        xt[:, q * 512 : (q + 1) * 512],
                        start=(c0 == 0),
                        stop=(c0 == bs - 2),
                        skip_group_check=True,
                    )
            nc.vector.tensor_copy(acc[:bs], psum[:bs, 0:C])
            for q in range(1, 4):
                nc.vector.tensor_add(
                    acc[:bs], acc[:bs], psum[:bs, q * C : (q + 1) * C]
                )

        # ---- epilogue: all `bs` batches at once, partition = batch ----
        s = acc
        stats = ep.tile([P, 2, 6], FP32)
        sv = s.rearrange("p (g d) -> p g d", g=2)
        for g in range(2):
            nc.vector.bn_stats(out=stats[:bs, g, :], in_=sv[:bs, g, :])
        mv = ep.tile([P, 2], FP32)
        nc.vector.bn_aggr(out=mv[:bs], in_=stats[:bs])
        mu = mv[:bs, 0:1]
        var = mv[:bs, 1:2]

        std = ep.tile([P, 1], FP32)
        nc.scalar.activation(
            out=std[:bs],
            in_=var,
            func=mybir.ActivationFunctionType.Sqrt,
            bias=eps_t[:bs],
            scale=1.0,
        )
        rstd = ep.tile([P, 1], FP32)
        nc.vector.reciprocal(out=rstd[:bs], in_=std[:bs])

        # wln = ln_w * W ; swln = sum(wln) ; c0 = sum(ln_b * W) + b
        wcopy = singles.tile([P, C], FP32)
        nc.vector.tensor_copy(wcopy, w_t)
        wln = singles.tile([P, C], FP32)
        nc.vector.tensor_mul(wln, lnw_t, wcopy)
        swln = ep.tile([P, 1], FP32)
        nc.vector.reduce_sum(out=swln, in_=wln, axis=mybir.AxisListType.X)
        scr0 = ep.tile([P, C], FP32)
        c0 = ep.tile([P, 1], FP32)
        nc.vector.tensor_mul(scr0, lnb_t, wcopy)
        nc.vector.reduce_sum(out=c0, in_=scr0, axis=mybir.AxisListType.X)
        nc.vector.tensor_add(c0, c0, b_t)

        # dot = s . wln  (per batch row)
        scr1 = ep.tile([P, C], FP32)
        dot = ep.tile([P, 1], FP32)
        nc.vector.tensor_mul(scr1[:bs], s[:bs], wln[:bs])
        nc.vector.reduce_sum(out=dot[:bs], in_=scr1[:bs], axis=mybir.AxisListType.X)

        # logits = rstd * (dot - mu * swln); out = sigmoid(logits + c0)
        t0 = ep.tile([P, 1], FP32)
        nc.vector.tensor_mul(t0[:bs], mu, swln[:bs])
        t1 = ep.tile([P, 1], FP32)
        nc.vector.tensor_sub(t1[:bs], dot[:bs], t0[:bs])
        t2 = ep.tile([P, 1], FP32)
        nc.vector.tensor_mul(t2[:bs], t1[:bs], rstd[:bs])
        res = ep.tile([P, 1], FP32)
        nc.scalar.activation(
            out=res[:bs],
            in_=t2[:bs],
            func=mybir.ActivationFunctionType.Sigmoid,
            bias=c0[:bs],
            scale=1.0,
        )
        nc.sync.dma_start(out=out[:, :], in_=res[:bs])

    nc.finalize()
    return nc


_NC_CACHE = {}


def kernel(**inputs) -> np.ndarray:
    global LAST_RESULT
    x = np.ascontiguousarray(np.asarray(inputs["x"], dtype=np.float32))
    ln_w = np.ascontiguousarray(np.asarray(inputs["ln_w"], dtype=np.float32))
    ln_b = np.ascontiguousarray(np.asarray(inputs["ln_b"], dtype=np.float32))
    W = np.ascontiguousarray(np.asarray(inputs["W"], dtype=np.float32))
    b = np.ascontiguousarray(np.asarray(inputs["b"], dtype=np.float32))

    if "full" not in _NC_CACHE:
        _NC_CACHE["full"] = build()
    nc = _NC_CACHE["full"]

    in_maps = [
        {
            "x": x[i * BS : (i + 1) * BS],
            "ln_w": ln_w,
            "ln_b": ln_b,
            "W": W,
            "b": b,
        }
        for i in range(NCORES)
    ]
    res = run_bass_kernel_spmd(nc, in_maps, list(range(NCORES)))
    LAST_RESULT = res
    return np.concatenate([res.results[i]["out"] for i in range(NCORES)], axis=0)
